# revision 47
# baseline (speedup 1.0000x reference)
"""Trainium2 Bass kernel for nn_EnsembleAdaptor: batched per-member MLP.

Per ensemble member (32 total): y = relu(x @ w1.T + b1) @ w2.T + b2
with x (512, 1024), w1 (4096, 1024), b1 (4096), w2 (1024, 4096), b2 (1024).

Sharding: pure data parallel over members — 4 members per core across 8 cores.

Device algorithm per member (all matmuls fp16 with fp32 PSUM accumulation;
fp16 runs at the same 1 cycle/row PE rate as bf16 but with 10 mantissa bits):
  layer 1 computes hT (H on partitions): for each j-tile (32 of them),
    accumulate 8 k-tiles of   psum[j,s] += w1T_tile.T @ xT_tile   then
    ScalarE relu(psum + b1) -> hT sbuf tile (fp16).
  layer 2 computes yT (DOUT on partitions): for each o-tile (8), accumulate
    32 k-tiles of   psum[o,s] += w2T_tile.T @ hT_tile,  then ScalarE
    identity(psum + b2) -> f32 sbuf -> DMA out as yT (contiguous).

Host side packs weights/activations into the exact SBUF layouts the PE
needs (contraction dim on partitions), so every DMA is contiguous.

The kernel is PE-bound: 1,048,576 matmul rows per core x 0.4167ns (2.4GHz)
= 437us floor; steady-state per-MM is 215.8ns = 512/2.4 + 2.5ns fixed NX
dispatch (irreducible at the N=512 PSUM-bank cap), so the real compute
floor is ~442us. Measured ~462.4us. The ~20us of fixed overhead:
  ~6.5us NEFF preamble before any engine dispatches;
  ~7.5us more until member-0's first chains can run hole-free — the two
    HWDGE queues (sync+scalar, the only fast DMA paths; gpsimd SWDGE is
    ~10x slower) wake at ~8.7/9.7us and sustain only ~165GB/s each, and
    member 0 needs w1(jt0,jt1) + all 2MB of x (~2.75MB) before chains can
    flow gap-free from ~14.2us — a hard supply bound;
  ~5us tail: final act + store issue/drain + ~2.9us fixed teardown
    (the teardown is ~0.35us shorter with the gpsimd/SWDGE path unused —
    biases ride the HWDGE queues instead).
HAM clock gate handling (this is what the 130 warmup matmuls are for):
the PE runs at 1.2GHz until a full free-running 4096-cycle activity
window (3.4-6.8us wall, phase-dependent) observes continuous busy; a PE
idle hole BEFORE that unthrottle resets the count (+ up to 3.4us of
half-speed chains — measured repeatedly), while a hole AFTER it is just
linear idle. 130 warmups = 6.9us of cold-rate busy guarantees the clock
is warm when they end for ANY window phase; the chains then start at the
supply bound with at most one benign post-warm wait. Schedule-fragility
warning from 10 measured iterations: the tile scheduler statically
interleaves instructions using its own DMA cost model, which is ~1-2us
optimistic at the head — designs that rely on data arriving between
warmup matmuls get holes planted mid-ramp and re-throttle. (fp8
DoubleRow was evaluated and rejected: real-HW throughput is only
~1.44-1.8x fp16, and e4m3 quantization error exceeds the 2e-2 gate even
for 2 of 8 k-tiles of layer 1 — measured 2.5e-2 — so neither full,
partial-contraction, nor residual-compensated fp8 wins.)
"""

import contextlib
import ctypes
import os
import sys
import types

import numpy as np
import ml_dtypes

import concourse.bass as bass
import concourse.tile as tile
from concourse import bacc, mybir
from concourse.bass_utils import run_bass_kernel_spmd


def _install_ntff_shim():
    """Provide antenv.axon_hooks + the ctypes NTFF profile hook when the
    image's antenv lacks them, so trace=True works under axon. Safe no-op
    on failure."""
    try:
        import antenv.axon_hooks  # noqa: F401
        return
    except ImportError:
        pass
    try:
        mod = types.ModuleType("antenv.axon_hooks")
        _state = {"hook": None}
        mod.set_axon_ntff_profile_hook = lambda h: _state.__setitem__("hook", h)
        mod.get_axon_ntff_profile_hook = lambda: _state["hook"]
        sys.modules["antenv.axon_hooks"] = mod
        import antenv
        antenv.axon_hooks = mod

        so_path = "/opt/axon/libaxon_pjrt.so"
        if not os.path.exists(so_path):
            return
        lib = ctypes.CDLL(so_path)
        if not hasattr(lib, "axon_start_nrt_profile"):
            return
        lib.axon_start_nrt_profile.argtypes = [
            ctypes.POINTER(ctypes.c_int64),
            ctypes.c_size_t,
        ]
        lib.axon_start_nrt_profile.restype = ctypes.c_int64
        lib.axon_stop_nrt_profile.argtypes = [ctypes.c_char_p]
        lib.axon_stop_nrt_profile.restype = ctypes.c_int64

        @contextlib.contextmanager
        def _hook(output_dir, device_ids):
            import jax
            jax.devices()
            if device_ids:
                ids = (ctypes.c_int64 * len(device_ids))(*device_ids)
                rc = lib.axon_start_nrt_profile(ids, len(device_ids))
            else:
                rc = lib.axon_start_nrt_profile(None, 0)
            if rc != 0:
                raise RuntimeError(f"axon_start_nrt_profile rc={rc}")
            try:
                yield
            finally:
                n = lib.axon_stop_nrt_profile(str(output_dir).encode())
                print(f"profile: {n} file(s) written to {output_dir}",
                      file=sys.stderr)

        mod.set_axon_ntff_profile_hook(_hook)
    except Exception:
        pass

B, S, DIN, H, DOUT = 32, 512, 1024, 4096, 1024
N_W1 = H * DIN
N_B1 = H
N_W2 = DOUT * H
N_B2 = DOUT

N_CORES = 8
M_PER = B // N_CORES  # members per core

DT = DIN // 128   # 8  k-tiles for layer 1
JT = H // 128     # 32 j-tiles (layer-1 outputs / layer-2 k-tiles)
OT = DOUT // 128  # 8  o-tiles for layer 2
SN = S            # 512 moving free dim

BF16 = mybir.dt.float16
F32 = mybir.dt.float32
NP_BF16 = np.float16

_cache = {}


def _build_nc():
    nc = bacc.Bacc("TRN2", target_bir_lowering=False, enable_partition_id=False)
    xp = nc.dram_tensor("xp", [M_PER, 128, DT * SN], BF16, kind="ExternalInput")
    w1p = nc.dram_tensor("w1p", [M_PER, JT, 128, DT * 128], BF16, kind="ExternalInput")
    w2p = nc.dram_tensor("w2p", [M_PER, OT, 128, JT * 128], BF16, kind="ExternalInput")
    b1p = nc.dram_tensor("b1p", [M_PER, 128, JT], F32, kind="ExternalInput")
    b2p = nc.dram_tensor("b2p", [M_PER, 128, OT], F32, kind="ExternalInput")
    ytp = nc.dram_tensor("ytp", [M_PER, OT, 128, SN], F32, kind="ExternalOutput")

    relu = mybir.ActivationFunctionType.Relu
    ident = mybir.ActivationFunctionType.Identity

    with tile.TileContext(nc) as tc:
        with (
            nc.sbuf_tensor([128, 192], BF16) as warm_t,
            tc.tile_pool(name="xpool", bufs=2) as xpool,
            tc.tile_pool(name="w1pool", bufs=6) as w1pool,
            tc.tile_pool(name="w2pool", bufs=4) as w2pool,
            tc.tile_pool(name="bpool", bufs=2) as bpool,
            tc.tile_pool(name="hpool", bufs=2) as hpool,
            tc.tile_pool(name="ypool", bufs=4) as ypool,
            tc.tile_pool(name="ps1", bufs=4, space="PSUM") as ps1pool,
            tc.tile_pool(name="ps2", bufs=4, space="PSUM") as ps2pool,
        ):
            # PE HAM warm-up. Measured model: the PE runs at 1.2GHz until
            # ONE FULL free-running 4096-cycle activity window (3.4us,
            # worst-case ~6.8us wall depending on phase) observes
            # continuous PE busy; a PE idle hole BEFORE that unthrottle
            # resets the count (costs multiple us), while a hole AFTER it
            # is just linear idle (MID re-throttle needs >3.4us idle).
            # So: burn 130 warmups = 6.9us of cold-rate PE busy starting
            # as early as possible, guaranteeing the clock is warm when
            # they end for ANY window phase, and let the real chains
            # start at the DMA supply bound with no other filler logic.
            # The warm tile is a RAW sbuf tensor read UNINITIALIZED
            # (garbage bf16 is fine: the product lands in a scratch PSUM
            # bank that is always overwritten later with start=True; raw
            # because the tile-pool release logic asserts on
            # read-but-never-written pool tiles). Dropping the memset
            # dependency starts PE busy ~0.6-1.0us earlier.
            warm_ps = ps2pool.tile([128, SN], F32, tag="ps2", name="ps2")
            for _ in range(126):
                nc.tensor.matmul(
                    warm_ps[:, 0:64], warm_t[:, 0:128], warm_t[:, 128:192],
                    start=True, stop=True,
                )
            # The last 4 warmups write a scratch tile on EACH of the 4
            # ps1 PSUM banks. Every real layer-1 chain's first matmul
            # then carries a WAR dependency on a late warmup, so the
            # static scheduler physically cannot hoist chain matmuls
            # into the ramp — the failure mode where its (optimistic)
            # DMA model plants a stalling matmul mid-warmup and the
            # resulting pre-unthrottle hole resets the HAM window.
            for _ in range(4):
                gate_ps = ps1pool.tile([128, SN], F32, tag="ps")
                nc.tensor.matmul(
                    gate_ps[:, 0:64], warm_t[:, 0:128], warm_t[:, 128:192],
                    start=True, stop=True,
                )

            for m in range(M_PER):
                x_t = xpool.tile([128, DT * SN], BF16)
                w1_first = w1pool.tile([128, DT * 128], BF16, tag="w1_t")
                # Member-0 head: the binding constraint (measured over
                # 12 runs) is DMA supply. The two HWDGE queues (sync
                # wakes ~8.7us, scalar ~9.7us) sustain ~165-220GB/s
                # each regardless of descriptor size, and chains need
                # w1(jt0) + all 1MB of x + w1(jt1) before they can flow
                # hole-free (~14.2us at best). Split across BOTH queues
                # in big pieces: w1 jt0 + x half A on sync; x half B +
                # b1 + w1 jt1 on scalar (jt1 must beat the second chain;
                # jt3/jt5 follow on scalar below). A single queue
                # serializes to ~16.5us. Heavy fragmentation (9+ small
                # pieces) regressed, and so did starting chains early on
                # finer-grained x sems (+3.1us): early chains interleave
                # supply stalls and every just-in-time chain pays ~170ns
                # of serialized LDWEIGHTS — let supply accumulate until
                # warmup-end, then run clean. Biases ride the scalar
                # queue (tiny) so the gpsimd/SWDGE path is never used
                # and its queue setup/teardown drop out of the epilogue.
                b1_t = bpool.tile([128, JT], F32, tag="b1")
                b2_t = bpool.tile([128, OT], F32, tag="b2")
                if m == 0:
                    # b1/b2 go on SYNC here: ahead of w1 jt=1 on scalar
                    # they delayed the jt=1 chain by ~0.9us (measured).
                    # (Splitting w1 jt0 into halves around xA to land xA
                    # ~0.55us earlier was tried and came out ~0.3us
                    # WORSE: the displaced sync pieces pushed the w1
                    # jt6+ supply later and opened a 0.9us stall at
                    # ~22us — the sync queue has no slack to re-order.)
                    # Unbalanced x halves: the first chain is gated by
                    # max(warmup-end, xA-sem), and xA lands up to 2us
                    # after warmup-end on slow-queue runs. Shrinking xA
                    # to 3 chunks (384KB) pulls that gate ~0.55us earlier
                    # WITHOUT displacing anything on sync (v14's mistake
                    # was adding pieces there); xB grows to 5 chunks and
                    # w1 jt=5 moves to sync to rebalance — every piece
                    # keeps >=1.4us of margin to its consumer chain.
                    nc.sync.dma_start(w1_first[:], w1p[m, 0])
                    nc.sync.dma_start(x_t[:, 0 : 3 * SN], xp[m, :, 0 : 3 * SN])
                    nc.scalar.dma_start(x_t[:, 3 * SN :], xp[m, :, 3 * SN :])
                    w1_jt1 = w1pool.tile([128, DT * 128], BF16, tag="w1_t")
                    nc.scalar.dma_start(w1_jt1[:], w1p[m, 1])
                    nc.sync.dma_start(b1_t[:], b1p[m])
                    nc.sync.dma_start(b2_t[:], b2p[m])
                else:
                    nc.sync.dma_start(w1_first[:], w1p[m, 0])
                    nc.sync.dma_start(x_t[:], xp[m])
                    nc.scalar.dma_start(b1_t[:], b1p[m])
                    nc.scalar.dma_start(b2_t[:], b2p[m])

                h_t = hpool.tile([128, JT * SN], BF16)
                for jt in range(JT):
                    if jt == 0:
                        w1_t = w1_first
                    elif m == 0 and jt == 1:
                        w1_t = w1_jt1
                    else:
                        w1_t = w1pool.tile([128, DT * 128], BF16, tag="w1_t")
                        if m == 0 and jt == 3:
                            nc.scalar.dma_start(w1_t[:], w1p[m, jt])
                        else:
                            nc.sync.dma_start(w1_t[:], w1p[m, jt])
                    ps = ps1pool.tile([128, SN], F32, tag="ps")
                    for k in range(DT):
                        nc.tensor.matmul(
                            ps[:],
                            w1_t[:, k * 128 : (k + 1) * 128],
                            x_t[:, k * SN : (k + 1) * SN],
                            start=(k == 0),
                            stop=(k == DT - 1),
                        )
                    nc.scalar.activation(
                        h_t[:, jt * SN : (jt + 1) * SN],
                        ps[:],
                        relu,
                        bias=b1_t[:, jt : jt + 1],
                    )

                for ot in range(OT):
                    w2_t = w2pool.tile([128, JT * 128], BF16)
                    nc.sync.dma_start(w2_t[:], w2p[m, ot])
                    if m == M_PER - 1 and ot == OT - 1:
                        # Last output tile: split into four 128-wide quarters
                        # so the earlier quarters' bias-add + store DMA overlap
                        # the later quarters' matmuls instead of serializing
                        # after the very last matmul.
                        for half in range(4):
                            lo = half * (SN // 4)
                            hi = lo + SN // 4
                            ps2 = ps2pool.tile([128, SN // 4], F32, tag="ps2")
                            for k in range(JT):
                                nc.tensor.matmul(
                                    ps2[:],
                                    w2_t[:, k * 128 : (k + 1) * 128],
                                    h_t[:, k * SN + lo : k * SN + hi],
                                    start=(k == 0),
                                    stop=(k == JT - 1),
                                )
                            y_t = ypool.tile([128, SN // 4], F32, tag="y_t")
                            nc.scalar.activation(
                                y_t[:], ps2[:], ident, bias=b2_t[:, ot : ot + 1]
                            )
                            # Split EVERY quarter store across both HWDGE
                            # queues: halves drain in parallel, and both
                            # queues stay awake through the last ~7us so
                            # the final 32KB halves skip the ~0.5-0.8us
                            # queue-wake-from-idle latency — the store is
                            # the last thing before fixed teardown.
                            mid = lo + SN // 8
                            nc.scalar.dma_start(
                                ytp[m, ot, :, lo:mid], y_t[:, 0 : SN // 8]
                            )
                            nc.sync.dma_start(
                                ytp[m, ot, :, mid:hi], y_t[:, SN // 8 :]
                            )
                        continue
                    ps2 = ps2pool.tile([128, SN], F32, tag="ps2")
                    for k in range(JT):
                        nc.tensor.matmul(
                            ps2[:],
                            w2_t[:, k * 128 : (k + 1) * 128],
                            h_t[:, k * SN : (k + 1) * SN],
                            start=(k == 0),
                            stop=(k == JT - 1),
                        )
                    y_t = ypool.tile([128, SN], F32, tag="y_t")
                    nc.scalar.activation(
                        y_t[:], ps2[:], ident, bias=b2_t[:, ot : ot + 1]
                    )
                    nc.scalar.dma_start(ytp[m, ot], y_t[:])
    nc.compile()
    return nc


def _pack_core(x_flat, ensemble_weights, members):
    """Pack one core's members into the DMA-friendly device layouts."""
    n = len(members)
    xp = np.empty((n, 128, DT * SN), dtype=NP_BF16)
    w1p = np.empty((n, JT, 128, DT * 128), dtype=NP_BF16)
    w2p = np.empty((n, OT, 128, JT * 128), dtype=NP_BF16)
    b1p = np.empty((n, 128, JT), dtype=np.float32)
    b2p = np.empty((n, 128, OT), dtype=np.float32)
    for i, mem in enumerate(members):
        x = x_flat[mem].reshape(S, DIN)
        o = 0
        w1 = ensemble_weights[mem, o : o + N_W1].reshape(H, DIN); o += N_W1
        b1 = ensemble_weights[mem, o : o + N_B1]; o += N_B1
        w2 = ensemble_weights[mem, o : o + N_W2].reshape(DOUT, H); o += N_W2
        b2 = ensemble_weights[mem, o : o + N_B2]
        # xp[p, dt*S + s] = x[s, dt*128+p]
        xp[i] = (
            x.reshape(S, DT, 128).transpose(2, 1, 0).reshape(128, DT * SN)
        ).astype(NP_BF16)
        # w1p[jt, p, dt*128+jj] = w1[jt*128+jj, dt*128+p]
        w1p[i] = (
            w1.reshape(JT, 128, DT, 128)
            .transpose(0, 3, 2, 1)
            .reshape(JT, 128, DT * 128)
        ).astype(NP_BF16)
        # w2p[ot, p, jt*128+oo] = w2[ot*128+oo, jt*128+p]
        w2p[i] = (
            w2.reshape(OT, 128, JT, 128)
            .transpose(0, 3, 2, 1)
            .reshape(OT, 128, JT * 128)
        ).astype(NP_BF16)
        b1p[i] = b1.reshape(JT, 128).T.astype(np.float32)
        b2p[i] = b2.reshape(OT, 128).T.astype(np.float32)
    return {"xp": xp, "w1p": w1p, "w2p": w2p, "b1p": b1p, "b2p": b2p}


def kernel(x_flat: np.ndarray, ensemble_weights: np.ndarray) -> np.ndarray:
    x_flat = np.asarray(x_flat, dtype=np.float32)
    ensemble_weights = np.asarray(ensemble_weights, dtype=np.float32)

    if "nc" not in _cache:
        _cache["nc"] = _build_nc()
    nc = _cache["nc"]

    in_maps = [
        _pack_core(x_flat, ensemble_weights,
                   list(range(c * M_PER, (c + 1) * M_PER)))
        for c in range(N_CORES)
    ]

    trace = bool(int(os.environ.get("KERNEL_TRACE", "0")))
    if trace:
        _install_ntff_shim()
    res = run_bass_kernel_spmd(nc, in_maps, core_ids=list(range(N_CORES)),
                               trace=trace)
    if trace:
        _cache["exec_time_ns"] = res.exec_time_ns

    out = np.empty((B, S * DOUT), dtype=np.float32)
    for c in range(N_CORES):
        ytp = res.results[c]["ytp"]  # (M_PER, OT, 128, SN)
        for i in range(M_PER):
            mem = c * M_PER + i
            # y[s, ot*128+p] = ytp[i, ot, p, s]
            out[mem] = (
                ytp[i].transpose(2, 0, 1).reshape(S * DOUT).astype(np.float32)
            )
    return out



# revision 48
# speedup vs baseline: 1.0011x; 1.0011x over previous
"""Trainium2 Bass kernel for nn_EnsembleAdaptor: batched per-member MLP.

Per ensemble member (32 total): y = relu(x @ w1.T + b1) @ w2.T + b2
with x (512, 1024), w1 (4096, 1024), b1 (4096), w2 (1024, 4096), b2 (1024).

Sharding: pure data parallel over members — 4 members per core across 8 cores.

Device algorithm per member (all matmuls fp16 with fp32 PSUM accumulation;
fp16 runs at the same 1 cycle/row PE rate as bf16 but with 10 mantissa bits):
  layer 1 computes hT (H on partitions): for each j-tile (32 of them),
    accumulate 8 k-tiles of   psum[j,s] += w1T_tile.T @ xT_tile   then
    ScalarE relu(psum + b1) -> hT sbuf tile (fp16).
  layer 2 computes yT (DOUT on partitions): for each o-tile (8), accumulate
    32 k-tiles of   psum[o,s] += w2T_tile.T @ hT_tile,  then ScalarE
    identity(psum + b2) -> f32 sbuf -> DMA out as yT (contiguous).

Host side packs weights/activations into the exact SBUF layouts the PE
needs (contraction dim on partitions), so every DMA is contiguous.

The kernel is PE-bound: 1,048,576 matmul rows per core x 0.4167ns (2.4GHz)
= 437us floor; steady-state per-MM is 215.8ns = 512/2.4 + 2.5ns fixed NX
dispatch (irreducible at the N=512 PSUM-bank cap), so the real compute
floor is ~442us. Measured ~462.4us. The ~20us of fixed overhead:
  ~6.5us NEFF preamble before any engine dispatches;
  ~7.5us more until member-0's first chains can run hole-free — the two
    HWDGE queues (sync+scalar, the only fast DMA paths; gpsimd SWDGE is
    ~10x slower) wake at ~8.7/9.7us and sustain only ~165GB/s each, and
    member 0 needs w1(jt0,jt1) + all 2MB of x (~2.75MB) before chains can
    flow gap-free from ~14.2us — a hard supply bound;
  ~5us tail: final act + store issue/drain + ~2.9us fixed teardown
    (the teardown is ~0.35us shorter with the gpsimd/SWDGE path unused —
    biases ride the HWDGE queues instead).
HAM clock gate handling (this is what the 130 warmup matmuls are for):
the PE runs at 1.2GHz until a full free-running 4096-cycle activity
window (3.4-6.8us wall, phase-dependent) observes continuous busy; a PE
idle hole BEFORE that unthrottle resets the count (+ up to 3.4us of
half-speed chains — measured repeatedly), while a hole AFTER it is just
linear idle. 130 warmups = 6.9us of cold-rate busy guarantees the clock
is warm when they end for ANY window phase; the chains then start at the
supply bound with at most one benign post-warm wait. Schedule-fragility
warning from 10 measured iterations: the tile scheduler statically
interleaves instructions using its own DMA cost model, which is ~1-2us
optimistic at the head — designs that rely on data arriving between
warmup matmuls get holes planted mid-ramp and re-throttle. (fp8
DoubleRow was evaluated and rejected: real-HW throughput is only
~1.44-1.8x fp16, and e4m3 quantization error exceeds the 2e-2 gate even
for 2 of 8 k-tiles of layer 1 — measured 2.5e-2 — so neither full,
partial-contraction, nor residual-compensated fp8 wins.)
"""

import contextlib
import ctypes
import os
import sys
import types

import numpy as np
import ml_dtypes

import concourse.bass as bass
import concourse.tile as tile
from concourse import bacc, mybir
from concourse.bass_utils import run_bass_kernel_spmd


def _install_ntff_shim():
    """Provide antenv.axon_hooks + the ctypes NTFF profile hook when the
    image's antenv lacks them, so trace=True works under axon. Safe no-op
    on failure."""
    try:
        import antenv.axon_hooks  # noqa: F401
        return
    except ImportError:
        pass
    try:
        mod = types.ModuleType("antenv.axon_hooks")
        _state = {"hook": None}
        mod.set_axon_ntff_profile_hook = lambda h: _state.__setitem__("hook", h)
        mod.get_axon_ntff_profile_hook = lambda: _state["hook"]
        sys.modules["antenv.axon_hooks"] = mod
        import antenv
        antenv.axon_hooks = mod

        so_path = "/opt/axon/libaxon_pjrt.so"
        if not os.path.exists(so_path):
            return
        lib = ctypes.CDLL(so_path)
        if not hasattr(lib, "axon_start_nrt_profile"):
            return
        lib.axon_start_nrt_profile.argtypes = [
            ctypes.POINTER(ctypes.c_int64),
            ctypes.c_size_t,
        ]
        lib.axon_start_nrt_profile.restype = ctypes.c_int64
        lib.axon_stop_nrt_profile.argtypes = [ctypes.c_char_p]
        lib.axon_stop_nrt_profile.restype = ctypes.c_int64

        @contextlib.contextmanager
        def _hook(output_dir, device_ids):
            import jax
            jax.devices()
            if device_ids:
                ids = (ctypes.c_int64 * len(device_ids))(*device_ids)
                rc = lib.axon_start_nrt_profile(ids, len(device_ids))
            else:
                rc = lib.axon_start_nrt_profile(None, 0)
            if rc != 0:
                raise RuntimeError(f"axon_start_nrt_profile rc={rc}")
            try:
                yield
            finally:
                n = lib.axon_stop_nrt_profile(str(output_dir).encode())
                print(f"profile: {n} file(s) written to {output_dir}",
                      file=sys.stderr)

        mod.set_axon_ntff_profile_hook(_hook)
    except Exception:
        pass

B, S, DIN, H, DOUT = 32, 512, 1024, 4096, 1024
N_W1 = H * DIN
N_B1 = H
N_W2 = DOUT * H
N_B2 = DOUT

N_CORES = 8
M_PER = B // N_CORES  # members per core

DT = DIN // 128   # 8  k-tiles for layer 1
JT = H // 128     # 32 j-tiles (layer-1 outputs / layer-2 k-tiles)
OT = DOUT // 128  # 8  o-tiles for layer 2
SN = S            # 512 moving free dim

BF16 = mybir.dt.float16
F32 = mybir.dt.float32
NP_BF16 = np.float16

_cache = {}


def _build_nc():
    nc = bacc.Bacc("TRN2", target_bir_lowering=False, enable_partition_id=False)
    xp = nc.dram_tensor("xp", [M_PER, 128, DT * SN], BF16, kind="ExternalInput")
    w1p = nc.dram_tensor("w1p", [M_PER, JT, 128, DT * 128], BF16, kind="ExternalInput")
    w2p = nc.dram_tensor("w2p", [M_PER, OT, 128, JT * 128], BF16, kind="ExternalInput")
    b1p = nc.dram_tensor("b1p", [M_PER, 128, JT], F32, kind="ExternalInput")
    b2p = nc.dram_tensor("b2p", [M_PER, 128, OT], F32, kind="ExternalInput")
    ytp = nc.dram_tensor("ytp", [M_PER, OT, 128, SN], F32, kind="ExternalOutput")

    relu = mybir.ActivationFunctionType.Relu
    ident = mybir.ActivationFunctionType.Identity

    with tile.TileContext(nc) as tc:
        with (
            nc.sbuf_tensor([128, 192], BF16) as warm_t,
            tc.tile_pool(name="xpool", bufs=2) as xpool,
            tc.tile_pool(name="w1pool", bufs=6) as w1pool,
            tc.tile_pool(name="w2pool", bufs=4) as w2pool,
            tc.tile_pool(name="bpool", bufs=2) as bpool,
            tc.tile_pool(name="hpool", bufs=2) as hpool,
            tc.tile_pool(name="ypool", bufs=4) as ypool,
            tc.tile_pool(name="ps1", bufs=4, space="PSUM") as ps1pool,
            tc.tile_pool(name="ps2", bufs=4, space="PSUM") as ps2pool,
        ):
            # PE HAM warm-up. Measured model: the PE runs at 1.2GHz until
            # ONE FULL free-running 4096-cycle activity window (3.4us,
            # worst-case ~6.8us wall depending on phase) observes
            # continuous PE busy; a PE idle hole BEFORE that unthrottle
            # resets the count (costs multiple us), while a hole AFTER it
            # is just linear idle (MID re-throttle needs >3.4us idle).
            # So: burn 130 warmups = 6.9us of cold-rate PE busy starting
            # as early as possible, guaranteeing the clock is warm when
            # they end for ANY window phase, and let the real chains
            # start at the DMA supply bound with no other filler logic.
            # The warm tile is a RAW sbuf tensor read UNINITIALIZED
            # (garbage bf16 is fine: the product lands in a scratch PSUM
            # bank that is always overwritten later with start=True; raw
            # because the tile-pool release logic asserts on
            # read-but-never-written pool tiles). Dropping the memset
            # dependency starts PE busy ~0.6-1.0us earlier.
            warm_ps = ps2pool.tile([128, SN], F32, tag="ps2", name="ps2")
            for _ in range(126):
                nc.tensor.matmul(
                    warm_ps[:, 0:64], warm_t[:, 0:128], warm_t[:, 128:192],
                    start=True, stop=True,
                )
            # The last 4 warmups write a scratch tile on EACH of the 4
            # ps1 PSUM banks. Every real layer-1 chain's first matmul
            # then carries a WAR dependency on a late warmup, so the
            # static scheduler physically cannot hoist chain matmuls
            # into the ramp — the failure mode where its (optimistic)
            # DMA model plants a stalling matmul mid-warmup and the
            # resulting pre-unthrottle hole resets the HAM window.
            for _ in range(4):
                gate_ps = ps1pool.tile([128, SN], F32, tag="ps")
                nc.tensor.matmul(
                    gate_ps[:, 0:64], warm_t[:, 0:128], warm_t[:, 128:192],
                    start=True, stop=True,
                )

            for m in range(M_PER):
                x_t = xpool.tile([128, DT * SN], BF16)
                w1_first = w1pool.tile([128, DT * 128], BF16, tag="w1_t")
                # Member-0 head: the binding constraint (measured over
                # 12 runs) is DMA supply. The two HWDGE queues (sync
                # wakes ~8.7us, scalar ~9.7us) sustain ~165-220GB/s
                # each regardless of descriptor size, and chains need
                # w1(jt0) + all 1MB of x + w1(jt1) before they can flow
                # hole-free (~14.2us at best). Split across BOTH queues
                # in big pieces: w1 jt0 + x half A on sync; x half B +
                # b1 + w1 jt1 on scalar (jt1 must beat the second chain;
                # jt3/jt5 follow on scalar below). A single queue
                # serializes to ~16.5us. Heavy fragmentation (9+ small
                # pieces) regressed, and so did starting chains early on
                # finer-grained x sems (+3.1us): early chains interleave
                # supply stalls and every just-in-time chain pays ~170ns
                # of serialized LDWEIGHTS — let supply accumulate until
                # warmup-end, then run clean. Biases ride the scalar
                # queue (tiny) so the gpsimd/SWDGE path is never used
                # and its queue setup/teardown drop out of the epilogue.
                b1_t = bpool.tile([128, JT], F32, tag="b1")
                b2_t = bpool.tile([128, OT], F32, tag="b2")
                if m == 0:
                    # b1/b2 go on SYNC here: ahead of w1 jt=1 on scalar
                    # they delayed the jt=1 chain by ~0.9us (measured).
                    # (Splitting w1 jt0 into halves around xA to land xA
                    # ~0.55us earlier was tried and came out ~0.3us
                    # WORSE: the displaced sync pieces pushed the w1
                    # jt6+ supply later and opened a 0.9us stall at
                    # ~22us — the sync queue has no slack to re-order.)
                    # x split 3/5 chunks across sync/scalar. Measured
                    # across 3 runs: the exact partition is NEUTRAL
                    # (462.02-462.07us for 3/5 and 4/4 alike) — the
                    # first chain is gated by max(warmup-end, slowest
                    # x-half sem) and shrinking one half just moves the
                    # gate to the other queue. The head supply integral
                    # (total bytes / two-queue rate from wake) is the
                    # invariant floor; do not spend more effort
                    # repartitioning it.
                    nc.sync.dma_start(w1_first[:], w1p[m, 0])
                    nc.sync.dma_start(x_t[:, 0 : 3 * SN], xp[m, :, 0 : 3 * SN])
                    nc.scalar.dma_start(x_t[:, 3 * SN :], xp[m, :, 3 * SN :])
                    w1_jt1 = w1pool.tile([128, DT * 128], BF16, tag="w1_t")
                    nc.scalar.dma_start(w1_jt1[:], w1p[m, 1])
                    nc.sync.dma_start(b1_t[:], b1p[m])
                    nc.sync.dma_start(b2_t[:], b2p[m])
                else:
                    nc.sync.dma_start(w1_first[:], w1p[m, 0])
                    nc.sync.dma_start(x_t[:], xp[m])
                    nc.scalar.dma_start(b1_t[:], b1p[m])
                    nc.scalar.dma_start(b2_t[:], b2p[m])

                h_t = hpool.tile([128, JT * SN], BF16)
                for jt in range(JT):
                    if jt == 0:
                        w1_t = w1_first
                    elif m == 0 and jt == 1:
                        w1_t = w1_jt1
                    else:
                        w1_t = w1pool.tile([128, DT * 128], BF16, tag="w1_t")
                        if m == 0 and jt == 3:
                            nc.scalar.dma_start(w1_t[:], w1p[m, jt])
                        else:
                            nc.sync.dma_start(w1_t[:], w1p[m, jt])
                    ps = ps1pool.tile([128, SN], F32, tag="ps")
                    for k in range(DT):
                        nc.tensor.matmul(
                            ps[:],
                            w1_t[:, k * 128 : (k + 1) * 128],
                            x_t[:, k * SN : (k + 1) * SN],
                            start=(k == 0),
                            stop=(k == DT - 1),
                        )
                    nc.scalar.activation(
                        h_t[:, jt * SN : (jt + 1) * SN],
                        ps[:],
                        relu,
                        bias=b1_t[:, jt : jt + 1],
                    )

                for ot in range(OT):
                    w2_t = w2pool.tile([128, JT * 128], BF16)
                    nc.sync.dma_start(w2_t[:], w2p[m, ot])
                    if m == M_PER - 1 and ot == OT - 1:
                        # Last output tile: split into four 128-wide quarters
                        # so the earlier quarters' bias-add + store DMA overlap
                        # the later quarters' matmuls instead of serializing
                        # after the very last matmul.
                        for half in range(4):
                            lo = half * (SN // 4)
                            hi = lo + SN // 4
                            ps2 = ps2pool.tile([128, SN // 4], F32, tag="ps2")
                            for k in range(JT):
                                nc.tensor.matmul(
                                    ps2[:],
                                    w2_t[:, k * 128 : (k + 1) * 128],
                                    h_t[:, k * SN + lo : k * SN + hi],
                                    start=(k == 0),
                                    stop=(k == JT - 1),
                                )
                            y_t = ypool.tile([128, SN // 4], F32, tag="y_t")
                            nc.scalar.activation(
                                y_t[:], ps2[:], ident, bias=b2_t[:, ot : ot + 1]
                            )
                            # Split EVERY quarter store across both HWDGE
                            # queues: halves drain in parallel, and both
                            # queues stay awake through the last ~7us so
                            # the final 32KB halves skip the ~0.5-0.8us
                            # queue-wake-from-idle latency — the store is
                            # the last thing before fixed teardown.
                            mid = lo + SN // 8
                            nc.scalar.dma_start(
                                ytp[m, ot, :, lo:mid], y_t[:, 0 : SN // 8]
                            )
                            nc.sync.dma_start(
                                ytp[m, ot, :, mid:hi], y_t[:, SN // 8 :]
                            )
                        continue
                    ps2 = ps2pool.tile([128, SN], F32, tag="ps2")
                    for k in range(JT):
                        nc.tensor.matmul(
                            ps2[:],
                            w2_t[:, k * 128 : (k + 1) * 128],
                            h_t[:, k * SN : (k + 1) * SN],
                            start=(k == 0),
                            stop=(k == JT - 1),
                        )
                    y_t = ypool.tile([128, SN], F32, tag="y_t")
                    nc.scalar.activation(
                        y_t[:], ps2[:], ident, bias=b2_t[:, ot : ot + 1]
                    )
                    nc.scalar.dma_start(ytp[m, ot], y_t[:])
    nc.compile()
    return nc


def _pack_core(x_flat, ensemble_weights, members):
    """Pack one core's members into the DMA-friendly device layouts."""
    n = len(members)
    xp = np.empty((n, 128, DT * SN), dtype=NP_BF16)
    w1p = np.empty((n, JT, 128, DT * 128), dtype=NP_BF16)
    w2p = np.empty((n, OT, 128, JT * 128), dtype=NP_BF16)
    b1p = np.empty((n, 128, JT), dtype=np.float32)
    b2p = np.empty((n, 128, OT), dtype=np.float32)
    for i, mem in enumerate(members):
        x = x_flat[mem].reshape(S, DIN)
        o = 0
        w1 = ensemble_weights[mem, o : o + N_W1].reshape(H, DIN); o += N_W1
        b1 = ensemble_weights[mem, o : o + N_B1]; o += N_B1
        w2 = ensemble_weights[mem, o : o + N_W2].reshape(DOUT, H); o += N_W2
        b2 = ensemble_weights[mem, o : o + N_B2]
        # xp[p, dt*S + s] = x[s, dt*128+p]
        xp[i] = (
            x.reshape(S, DT, 128).transpose(2, 1, 0).reshape(128, DT * SN)
        ).astype(NP_BF16)
        # w1p[jt, p, dt*128+jj] = w1[jt*128+jj, dt*128+p]
        w1p[i] = (
            w1.reshape(JT, 128, DT, 128)
            .transpose(0, 3, 2, 1)
            .reshape(JT, 128, DT * 128)
        ).astype(NP_BF16)
        # w2p[ot, p, jt*128+oo] = w2[ot*128+oo, jt*128+p]
        w2p[i] = (
            w2.reshape(OT, 128, JT, 128)
            .transpose(0, 3, 2, 1)
            .reshape(OT, 128, JT * 128)
        ).astype(NP_BF16)
        b1p[i] = b1.reshape(JT, 128).T.astype(np.float32)
        b2p[i] = b2.reshape(OT, 128).T.astype(np.float32)
    return {"xp": xp, "w1p": w1p, "w2p": w2p, "b1p": b1p, "b2p": b2p}


def kernel(x_flat: np.ndarray, ensemble_weights: np.ndarray) -> np.ndarray:
    x_flat = np.asarray(x_flat, dtype=np.float32)
    ensemble_weights = np.asarray(ensemble_weights, dtype=np.float32)

    if "nc" not in _cache:
        _cache["nc"] = _build_nc()
    nc = _cache["nc"]

    in_maps = [
        _pack_core(x_flat, ensemble_weights,
                   list(range(c * M_PER, (c + 1) * M_PER)))
        for c in range(N_CORES)
    ]

    trace = bool(int(os.environ.get("KERNEL_TRACE", "0")))
    if trace:
        _install_ntff_shim()
    res = run_bass_kernel_spmd(nc, in_maps, core_ids=list(range(N_CORES)),
                               trace=trace)
    if trace:
        _cache["exec_time_ns"] = res.exec_time_ns

    out = np.empty((B, S * DOUT), dtype=np.float32)
    for c in range(N_CORES):
        ytp = res.results[c]["ytp"]  # (M_PER, OT, 128, SN)
        for i in range(M_PER):
            mem = c * M_PER + i
            # y[s, ot*128+p] = ytp[i, ot, p, s]
            out[mem] = (
                ytp[i].transpose(2, 0, 1).reshape(S * DOUT).astype(np.float32)
            )
    return out

